# revision 1
# baseline (speedup 1.0000x reference)
"""Trainium2 Bass kernel for the Nonlocal(Linear-embed) block.

Sharding: 8 cores = 4 samples x 2 halves of the `cd` (second spatial-flat)
axis of the [HW, HW] affinity matrix. Per core:
  - g.T half   = (l @ W_lin.T).T rows for its cd-half     (PE, W streamed)
  - pass A     : colsum[cd] = sum_ab exp(f[ab,cd]) for its cd-half (f = theta^T theta,
                 regenerated on the fly; exp on ACT with fused accumulate)
  - pass B     : e2 = exp(exp(f - ln colsum))  -> partial out = G'^T e2 summed over
                 its cd-half, with a ones-column in G' producing the second-softmax
                 row-sum for free
  - pair AllReduce of [65, HW] partial; normalize; depthwise 3x3 conv;
    8-core AllReduce of BN stats; BN + residual.
Both softmax max-subtractions are skipped (f is bounded by max ||theta||^2 ~ 50,
exp stays finite in fp32; second softmax input is in (0,1]).
"""

import numpy as np

import concourse.bacc as bacc
import concourse.bass as bass
import concourse.mybir as mybir
import concourse.tile as tile
from concourse import bass_utils

N, C, H, W = 4, 64, 56, 56
HW = H * W            # 3136
CR = 16               # reduced channel count (0.25 * C)
HALF = HW // 2        # 1568
PT = 112              # partition-tile size: 3136 = 28*112, 1568 = 14*112
NKT = HW // PT        # 28 k-tiles
NCT = HALF // PT      # 14 cd-tiles per half
NCORES = 8
BN_EPS = 1e-5
NB = 4                # free-dim chunks per half (1568 = 4*392)
CB = HALF // NB       # 392
NA = 7                # free-dim chunks for pass A (3136 = 7*448)
AB = HW // NA         # 448

F32 = mybir.dt.float32
AF = mybir.ActivationFunctionType
ALU = mybir.AluOpType

_CACHE = {}


def build_nc(dbg=False):
    nc = bacc.Bacc(
        "TRN2", target_bir_lowering=False, debug=False, num_devices=NCORES
    )

    theta_d = nc.dram_tensor("theta", [CR, HW], F32, kind="ExternalInput")
    lT_d = nc.dram_tensor("lT", [HW, C], F32, kind="ExternalInput")
    # host-prepped W_lin[cd_half, :].T  -> [HW(k), HALF(cd)]
    wt_d = nc.dram_tensor("wt", [HW, HALF], F32, kind="ExternalInput")
    # host-prepped theta[:, cd_half]  (cd-side slice of theta)
    thcd_d = nc.dram_tensor("thcd", [CR, HALF], F32, kind="ExternalInput")
    lres_d = nc.dram_tensor("lres", [C, HW], F32, kind="ExternalInput")
    dw_d = nc.dram_tensor("dw", [C, 9], F32, kind="ExternalInput")
    bnw_d = nc.dram_tensor("bnw", [C, 1], F32, kind="ExternalInput")
    bnb_d = nc.dram_tensor("bnb", [C, 1], F32, kind="ExternalInput")
    eye_d = nc.dram_tensor("eye", [C, C], F32, kind="ExternalInput")
    y_d = nc.dram_tensor("y", [C, HW], F32, kind="ExternalOutput")
    if dbg:
        dbg_d = {
            "d_gnat": nc.dram_tensor("d_gnat", [C, HALF], F32, kind="ExternalOutput"),
            "d_colsum": nc.dram_tensor("d_colsum", [PT, NCT], F32, kind="ExternalOutput"),
            "d_outacc": nc.dram_tensor("d_outacc", [C + 1, HW], F32, kind="ExternalOutput"),
            "d_red": nc.dram_tensor("d_red", [C + 1, HW], F32, kind="ExternalOutput"),
            "d_normed": nc.dram_tensor("d_normed", [C, HW], F32, kind="ExternalOutput"),
            "d_conv": nc.dram_tensor("d_conv", [C, HW], F32, kind="ExternalOutput"),
        }

    with tile.TileContext(nc) as tc:
        with (
            tc.tile_pool(name="const", bufs=1) as constp,
            tc.tile_pool(name="kslab", bufs=3) as kslabp,
            tc.tile_pool(name="gsb", bufs=NCT) as gsbp,
            tc.tile_pool(name="work", bufs=3) as workp,
            tc.tile_pool(name="e2", bufs=2) as e2p,
            tc.tile_pool(name="big", bufs=1) as bigp,
            tc.tile_pool(name="psA", bufs=2, space=bass.MemorySpace.PSUM) as psA,
            tc.tile_pool(name="psT", bufs=2, space=bass.MemorySpace.PSUM) as psT,
            tc.tile_pool(name="psBig", bufs=1, space=bass.MemorySpace.PSUM) as psBig,
            tc.tile_pool(name="dram", bufs=1, space=bass.MemorySpace.DRAM) as dramp,
        ):
            # ---- constants / small inputs ----
            theta_sb = constp.tile([CR, HW], F32)
            nc.sync.dma_start(theta_sb[:], theta_d.ap())
            thcd_sb = constp.tile([CR, HALF], F32)
            nc.sync.dma_start(thcd_sb[:], thcd_d.ap())
            lT_sb = constp.tile([PT, NKT, C], F32)
            nc.sync.dma_start(
                lT_sb[:], lT_d.ap().rearrange("(a p) c -> p a c", p=PT)
            )
            eye_sb = constp.tile([C, C], F32)
            nc.sync.dma_start(eye_sb[:], eye_d.ap())
            dw_sb = constp.tile([C, 9], F32)
            nc.sync.dma_start(dw_sb[:], dw_d.ap())
            bnw_sb = constp.tile([C, 1], F32)
            nc.sync.dma_start(bnw_sb[:], bnw_d.ap())
            bnb_sb = constp.tile([C, 1], F32)
            nc.sync.dma_start(bnb_sb[:], bnb_d.ap())
            lres_sb = constp.tile([C, HW], F32)
            nc.sync.dma_start(lres_sb[:], lres_d.ap())

            # ---- g phase: g_nat[c, cd] = sum_k l[c, k] W[cd, k] for my cd-half
            # kslab_kt[p, cd] = W.T[kt*PT + p, cd]  (contiguous rows of wt)
            wtT = wt_d.ap().rearrange("(a p) c -> p a c", p=PT)  # [PT, NKT, HALF]
            # PSUM: one bank (512 f32) per 392-col chunk so no matmul output
            # crosses a bank boundary
            gnat_ps = psBig.tile([C, NB, 512], F32, tag="bigps")
            for kt in range(NKT):
                kslab = kslabp.tile([PT, HALF], F32)
                nc.sync.dma_start(kslab[:], wtT[:, kt, :])
                for j in range(NB):
                    nc.tensor.matmul(
                        gnat_ps[:, j, 0:CB],
                        lT_sb[:, kt, :],
                        kslab[:, j * CB:(j + 1) * CB],
                        start=(kt == 0),
                        stop=(kt == NKT - 1),
                    )
            gnat_sb = bigp.tile([C, HALF], F32)
            nc.vector.tensor_copy(
                gnat_sb[:].rearrange("p (j c) -> p j c", c=CB),
                gnat_ps[:, :, 0:CB],
            )

            # transpose to g.T tiles [PT, C], augmented with a ones column
            gsb = []
            for t in range(NCT):
                pt = psT.tile([PT, C], F32)
                nc.tensor.transpose(
                    pt[:], gnat_sb[:, t * PT:(t + 1) * PT], eye_sb[:]
                )
                g = gsbp.tile([PT, C + 1], F32)
                nc.vector.tensor_copy(g[:, 0:C], pt[:])
                nc.vector.memset(g[:, C:C + 1], 1.0)
                gsb.append(g)

            # ---- pass A: colsum[cd] = sum_ab exp(f[ab, cd]), cd in my half ----
            colsum = constp.tile([PT, NCT], F32)
            for t in range(NCT):
                th_t = thcd_sb[:, t * PT:(t + 1) * PT]
                colacc = workp.tile([PT, NA], F32)
                for j in range(NA):
                    fA = psA.tile([PT, AB], F32, tag="fps")
                    nc.tensor.matmul(
                        fA[:],
                        th_t,
                        theta_sb[:, j * AB:(j + 1) * AB],
                        start=True,
                        stop=True,
                    )
                    scr = workp.tile([PT, AB], F32)
                    nc.scalar.activation(
                        scr[:], fA[:], AF.Exp, accum_out=colacc[:, j:j + 1]
                    )
                nc.vector.reduce_sum(
                    colsum[:, t:t + 1], colacc[:], axis=mybir.AxisListType.X
                )
            if dbg:
                nc.sync.dma_start(dbg_d["d_gnat"].ap(), gnat_sb[:])
                nc.sync.dma_start(dbg_d["d_colsum"].ap(), colsum[:])
            nlncs = constp.tile([PT, NCT], F32)
            nc.scalar.activation(nlncs[:], colsum[:], AF.Ln)
            nc.vector.tensor_scalar_mul(nlncs[:], nlncs[:], -1.0)

            # ---- pass B + out accumulation ----
            out_acc = bigp.tile([C + 1, HW], F32)
            for a in range(2):
                outp = psBig.tile([C + 1, NB, 512], F32, tag="bigps")
                for t in range(NCT):
                    th_t = thcd_sb[:, t * PT:(t + 1) * PT]
                    s1 = e2p.tile([PT, HALF], F32)
                    for j in range(NB):
                        fB = psA.tile([PT, AB], F32, tag="fps")
                        nc.tensor.matmul(
                            fB[:, 0:CB],
                            th_t,
                            theta_sb[:, a * HALF + j * CB: a * HALF + (j + 1) * CB],
                            start=True,
                            stop=True,
                        )
                        nc.scalar.activation(
                            s1[:, j * CB:(j + 1) * CB],
                            fB[:, 0:CB],
                            AF.Exp,
                            bias=nlncs[:, t:t + 1],
                        )
                    e2 = e2p.tile([PT, HALF], F32)
                    nc.scalar.activation(e2[:], s1[:], AF.Exp)
                    for j in range(NB):
                        nc.tensor.matmul(
                            outp[:, j, 0:CB],
                            gsb[t][:],
                            e2[:, j * CB:(j + 1) * CB],
                            start=(t == 0),
                            stop=(t == NCT - 1),
                        )
                nc.vector.tensor_copy(
                    out_acc[:, a * HALF:(a + 1) * HALF].rearrange(
                        "p (j c) -> p j c", c=CB
                    ),
                    outp[:, :, 0:CB],
                )

            # ---- pair AllReduce of [C+1, HW] partial sums ----
            if dbg:
                nc.sync.dma_start(dbg_d["d_outacc"].ap(), out_acc[:])
            ar_in = dramp.tile([C + 1, HW], F32)
            ar_out = dramp.tile([C + 1, HW], F32)
            nc.sync.dma_start(ar_in[:], out_acc[:])
            nc.gpsimd.collective_compute(
                "AllReduce",
                ALU.add,
                replica_groups=[[0, 1], [2, 3], [4, 5], [6, 7]],
                ins=[ar_in.opt()],
                outs=[ar_out.opt()],
            )
            red = bigp.tile([C + 1, HW], F32, tag="out_acc")
            nc.sync.dma_start(red[:], ar_out[:])

            # ---- normalize by second-softmax row sums ----
            rec = constp.tile([1, HW], F32)
            nc.vector.reciprocal(rec[:], red[C:C + 1, :])
            bc = bigp.tile([C, HW], F32)
            nc.gpsimd.partition_broadcast(bc[:], rec[:], channels=C)
            normed = bigp.tile([C, HW], F32)
            nc.vector.tensor_mul(normed[:], red[0:C, :], bc[:])

            # ---- depthwise 3x3 conv, padding 1 (full sample; host keeps a half) ----
            if dbg:
                nc.sync.dma_start(dbg_d["d_red"].ap(), red[:])
                nc.sync.dma_start(dbg_d["d_normed"].ap(), normed[:])
            pad = bigp.tile([C, H + 2, W + 2], F32)
            nc.vector.memset(pad[:], 0.0)
            nc.vector.tensor_copy(
                pad[:, 1:H + 1, 1:W + 1],
                normed[:].rearrange("p (r c) -> p r c", c=W),
            )
            ca = bigp.tile([C, H, W], F32)
            cb = bigp.tile([C, H, W], F32)
            nc.vector.tensor_scalar_mul(
                ca[:], pad[:, 0:H, 0:W], dw_sb[:, 0:1]
            )
            cur, other = ca, cb
            for tap in range(1, 9):
                dy, dx = divmod(tap, 3)
                nc.vector.scalar_tensor_tensor(
                    other[:],
                    pad[:, dy:dy + H, dx:dx + W],
                    dw_sb[:, tap:tap + 1],
                    cur[:],
                    op0=ALU.mult,
                    op1=ALU.add,
                )
                cur, other = other, cur
            conv = cur  # 9 taps -> ends in ca

            if dbg:
                nc.sync.dma_start(
                    dbg_d["d_conv"].ap().rearrange("p (r c) -> p r c", c=W),
                    conv[:],
                )
            # ---- BN stats (partial) + 8-core AllReduce ----
            st = constp.tile([C, 2], F32)
            nc.vector.reduce_sum(
                st[:, 0:1], conv[:], axis=mybir.AxisListType.XY
            )
            sq = bigp.tile([C, H, W], F32, tag="cb")
            nc.scalar.activation(
                sq[:], conv[:], AF.Square, accum_out=st[:, 1:2]
            )
            bn_in = dramp.tile([C, 2], F32)
            bn_out = dramp.tile([C, 2], F32)
            nc.sync.dma_start(bn_in[:], st[:])
            nc.gpsimd.collective_compute(
                "AllReduce",
                ALU.add,
                replica_groups=[list(range(NCORES))],
                ins=[bn_in.opt()],
                outs=[bn_out.opt()],
            )
            str_ = constp.tile([C, 2], F32)
            nc.sync.dma_start(str_[:], bn_out[:])

            # each sample's stats counted twice (both pair cores) -> 2*N*H*W
            inv_n = 1.0 / (2.0 * N * H * W)
            mu = constp.tile([C, 1], F32)
            nc.vector.tensor_scalar_mul(mu[:], str_[:, 0:1], inv_n)
            m2 = constp.tile([C, 1], F32)
            nc.vector.tensor_scalar_mul(m2[:], str_[:, 1:2], inv_n)
            mu2 = constp.tile([C, 1], F32)
            nc.scalar.square(mu2[:], mu[:])
            var = constp.tile([C, 1], F32)
            nc.vector.tensor_sub(var[:], m2[:], mu2[:])
            nc.vector.tensor_scalar_add(var[:], var[:], BN_EPS)
            std = constp.tile([C, 1], F32)
            nc.scalar.sqrt(std[:], var[:])
            inv = constp.tile([C, 1], F32)
            nc.vector.reciprocal(inv[:], std[:])
            scl = constp.tile([C, 1], F32)
            nc.vector.tensor_mul(scl[:], inv[:], bnw_sb[:])
            tmp = constp.tile([C, 1], F32)
            nc.vector.tensor_mul(tmp[:], mu[:], scl[:])
            bia = constp.tile([C, 1], F32)
            nc.vector.tensor_sub(bia[:], bnb_sb[:], tmp[:])

            # ---- y = conv*scl + bia + l ----
            yt = bigp.tile([C, H, W], F32, tag="normed")
            nc.vector.tensor_scalar(
                yt[:], conv[:], scl[:], bia[:], op0=ALU.mult, op1=ALU.add
            )
            yt2 = bigp.tile([C, H, W], F32, tag="bc")
            nc.vector.tensor_add(
                yt2[:], yt[:], lres_sb[:].rearrange("p (r c) -> p r c", c=W)
            )
            nc.sync.dma_start(
                y_d.ap().rearrange("p (r c) -> p r c", c=W), yt2[:]
            )

    nc.compile()
    return nc


def _prep_inputs(l, W_lin, dw_kernel, bn_weight, bn_bias):
    l = np.asarray(l, dtype=np.float32)
    W_lin = np.asarray(W_lin, dtype=np.float32)
    dw = np.ascontiguousarray(
        np.asarray(dw_kernel, dtype=np.float32).reshape(C, 9)
    )
    bnw = np.ascontiguousarray(
        np.asarray(bn_weight, dtype=np.float32).reshape(C, 1)
    )
    bnb = np.ascontiguousarray(
        np.asarray(bn_bias, dtype=np.float32).reshape(C, 1)
    )
    eye = np.eye(C, dtype=np.float32)
    wtT = [
        np.ascontiguousarray(W_lin[h * HALF:(h + 1) * HALF, :].T)
        for h in range(2)
    ]
    in_maps = []
    for c in range(NCORES):
        n, h = divmod(c, 2)
        ln = np.ascontiguousarray(l[n].reshape(C, HW))
        in_maps.append(
            {
                "theta": np.ascontiguousarray(ln[:CR]),
                "thcd": np.ascontiguousarray(ln[:CR, h * HALF:(h + 1) * HALF]),
                "lT": np.ascontiguousarray(ln.T),
                "wt": wtT[h],
                "lres": ln,
                "dw": dw,
                "bnw": bnw,
                "bnb": bnb,
                "eye": eye,
            }
        )
    return in_maps


def kernel(l, W_lin, dw_kernel, bn_weight, bn_bias):
    if "nc" not in _CACHE:
        _CACHE["nc"] = build_nc()
    nc = _CACHE["nc"]
    in_maps = _prep_inputs(l, W_lin, dw_kernel, bn_weight, bn_bias)
    res = bass_utils.run_bass_kernel_spmd(
        nc, in_maps, core_ids=list(range(NCORES))
    )
    out = np.empty((N, C, H, W), np.float32)
    for c in range(NCORES):
        n, h = divmod(c, 2)
        full = res.results[c]["y"].reshape(C, H, W)
        out[n, :, h * 28:(h + 1) * 28, :] = full[:, h * 28:(h + 1) * 28, :]
    return out

